# revision 26
# baseline (speedup 1.0000x reference)
"""Trainium2 Bass kernel for the batched kinematics layer.

Math:
  Per batch element b: root transform Tg(qpos[b,0:6]); then per chain c the
  sequential composition T <- T @ (P0[c,j] + sin(q)*P1 + cos(q)*P2) where
  P0/P1/P2 are constant 4x4s precomputed on host from offsets/axes.  The
  per-link vertex transform pts = R@v + t is one matmul per link with
  contraction K=12: out[b, (v,x)] = sum_k A[k,b] * W[k,(v,x)], A = transposed
  link-transform entries (k = x*4+l), W built on host from verts.

  The root rotation avoids sqrt / trig entirely: R = I + a*K + b*K^2 with
  K = skew(aa) unnormalized, a = sin(t)/t and b = (1-cos(t))/t^2 evaluated
  as degree-4 Horner polynomials in s2 = t^2 (~7e-4 abs err on the input
  range, far under the 2e-2 gate).  Joint sin/cos go through the ScalarE
  Sin LUT after a one-op add_range_wrap range reduction (inputs < 3*pi, so
  a single-period wrap is exact).  A and W are plain bf16 (bf16 ~2.6e-3).

  The kernel is memory-bound: 62.9 MB of f32 output per core at ~360 GB/s
  (~175 us).  Everything else exists to start the output-write DMA stream
  early and never let it stall:
    - batch-tile-0 work sits in a high-priority band; its first chain
      ships per-PSUM-bank (256 KB DMAs), the rest per-link (768 KB);
    - chain-0 weights load in a small first DMA so the first matmuls are
      not blocked on the full weight load in the Tile scheduler's model;
    - M-matrix builds run on the otherwise idle Pool engine, keeping the
      DVE free for the serial root/compose dependency chain;
    - batch tiles 1-3 are data-gated behind bt0's last compose so their
      DVE work cannot interleave into the critical chain;
    - idle-priority dummy transposes keep the PE clock ramped so the
      first real matmuls run at full speed.

Sharding: pure data-parallel over batch, 8 cores x 512 batch elements.
"""
import math
import numpy as np
from contextlib import ExitStack

import concourse.bass as bass
import concourse.mybir as mybir
import concourse.tile as tile
from concourse import bacc
from concourse.bass_utils import run_bass_kernel_spmd
from concourse.masks import make_identity

F32 = mybir.dt.float32
BF16 = mybir.dt.bfloat16
AX = mybir.AxisListType
OP = mybir.AluOpType
AF = mybir.ActivationFunctionType

N_CHAINS, N_JOINTS, N_VERTS = 5, 4, 512
NLINK = N_CHAINS * N_JOINTS          # 20
VX = N_VERTS * 3                     # 1536
ROW = NLINK * VX                     # 30720
B_FULL = 4096
N_CORES = 8
B_CORE = B_FULL // N_CORES           # 512
P = 128
NB = B_CORE // P                     # 4 batch tiles per core
TWO_PI = float(np.float32(2.0 * math.pi))
PI = float(np.float32(math.pi))
POLY_DEG = 4
NCOEF = 2 * (POLY_DEG + 1)           # interleaved (ca_k, cb_k) high->low
PCON_LEN = 960 + NCOEF

MM_MODE = "bf16"
REPEAT = 1
HIPRI = 1 << 20

# tuning knobs (sim-swept)
KNOBS = {
    "a12_eng": "scalar",     # engine for PSUM->SBUF A12 evacuation
    "pat_c0": (0, 1),        # copy engine pattern for (bt0, c0): 1=vector
    "pat_rest": (1, 0, 0, 1, 0),
    "psA": 2,
    "psO": 4,
    "gate": False,
    "wsplit": (0, 1, 4),     # link boundaries for split weight loads
    "warm": 32,              # PE clock warm-up dummy transposes
    "gran": "link",          # output DMA granularity for non-first chains
    "c0gran": "bank",        # first chain: "bank" or fall back to gran
    "m_eng": "pool",         # engine for M-builds
    "warm_loop": 24,         # per-iteration PE warm-up (loop builds only)
}


def _view(t, off, dims):
    """Custom free-dim view of a tile AP: keep partition pair, replace free dims."""
    ap = [list(t.ap[0])] + [[s, c] for (s, c) in dims]
    return bass.AP(t.tensor, t.offset + off, ap)


def _sinc_coeffs():
    """Power-basis fits of a(x)=sin(sqrt x)/sqrt x and b(x)=(1-cos(sqrt x))/x
    on x in [0, 26], high->low for Horner."""
    from numpy.polynomial import chebyshev as C
    x = np.linspace(0, 26.0, 50001)
    th = np.sqrt(x)
    a = np.where(th > 1e-12, np.sin(th) / np.maximum(th, 1e-300), 1.0)
    b = np.where(x > 1e-12, (1 - np.cos(th)) / np.maximum(x, 1e-300), 0.5)
    ca = C.cheb2poly(C.chebfit(x, a, POLY_DEG))[::-1]
    cb = C.cheb2poly(C.chebfit(x, b, POLY_DEG))[::-1]
    out = np.empty(NCOEF, np.float32)
    out[0::2] = ca
    out[1::2] = cb
    return out


def _host_constants(offsets, axes, verts):
    off = offsets.astype(np.float64)
    ax = axes.astype(np.float64)
    K = np.zeros((N_CHAINS, N_JOINTS, 4, 4))
    x, y, z = ax[..., 0], ax[..., 1], ax[..., 2]
    K[..., 0, 1] = -z; K[..., 0, 2] = y
    K[..., 1, 0] = z;  K[..., 1, 2] = -x
    K[..., 2, 0] = -y; K[..., 2, 1] = x
    K2 = K @ K
    offK = off @ K
    offK2 = off @ K2
    pcon = np.stack([off + offK2, offK, -offK2], 0).reshape(3, NLINK, 16)
    pcon = np.ascontiguousarray(pcon, np.float32)

    W = np.zeros((12, NLINK, VX), np.float32)
    vv = verts.reshape(NLINK, N_VERTS, 3)
    for xx in range(3):
        for l in range(3):
            W[xx * 4 + l, :, xx::3] = vv[:, :, l]
        W[xx * 4 + 3, :, xx::3] = 1.0
    return pcon, W


def _build_nc_bf16(repeat):
    nc = bacc.Bacc("TRN2", target_bir_lowering=False, debug=False)

    qpos = nc.dram_tensor("qpos", [B_CORE, 26], F32, kind="ExternalInput")
    pcon = nc.dram_tensor("pcon", [PCON_LEN], F32, kind="ExternalInput")
    wmat = nc.dram_tensor("wmat", [12, NLINK * VX], BF16, kind="ExternalInput")
    out = nc.dram_tensor("out", [B_CORE, ROW], F32, kind="ExternalOutput")

    with tile.TileContext(nc) as tc, ExitStack() as ctx:
        const = ctx.enter_context(tc.tile_pool(name="const", bufs=1))
        qp_pool = ctx.enter_context(tc.tile_pool(name="qp", bufs=2))
        small = ctx.enter_context(tc.tile_pool(name="small", bufs=2))
        tpool = ctx.enter_context(tc.tile_pool(name="tpool", bufs=6))
        mpool = ctx.enter_context(tc.tile_pool(name="mpool", bufs=2))
        apool = ctx.enter_context(tc.tile_pool(name="apool", bufs=8))
        ostage = ctx.enter_context(tc.tile_pool(name="ostage", bufs=3))
        psA = ctx.enter_context(tc.tile_pool(name="psA", bufs=KNOBS["psA"], space="PSUM"))
        psO = ctx.enter_context(tc.tile_pool(name="psO", bufs=KNOBS["psO"], space="PSUM"))

        # ---- act-table preloads (Sin + Copy sets) and constants ----
        zero_c = const.tile([P, 1], F32, name="zero_c")
        nc.vector.memset(zero_c, 0.0)
        dummy = const.tile([P, 1], F32, name="dummy")
        nc.scalar.activation(dummy, zero_c, AF.Sin, bias=zero_c)
        dummy2 = const.tile([P, 1], F32, name="dummy2")
        nc.scalar.copy(dummy2, zero_c)

        pt = const.tile([P, PCON_LEN], F32, name="pt")
        nc.gpsimd.dma_start(out=pt, in_=bass.AP(pcon, 0, [[0, P], [1, PCON_LEN]]))

        w_sb = const.tile([12, NLINK * VX], BF16, name="w_sb")
        ident = const.tile([P, P], F32, name="ident")
        make_identity(nc, ident)
        psD = None
        if KNOBS.get("warm") or KNOBS.get("warm_loop"):
            psD = ctx.enter_context(tc.tile_pool(name="psD", bufs=2, space="PSUM"))
        # low-priority dummy transposes: keep the PE clock ramped through
        # the head so the first real matmuls run at full speed
        for _ in range(KNOBS.get("warm", 0)):
            wps = psD.tile([P, P], F32, name="wps", space="PSUM")
            nc.tensor.transpose(wps, ident, ident)

        def load_w():
            # chain-0 weights first in small DMAs so c0 matmuls unblock early
            wb = [b * VX for b in KNOBS["wsplit"]] + [NLINK * VX]
            for si in range(len(wb) - 1):
                lo, hi = wb[si], wb[si + 1]
                nc.sync.dma_start(
                    out=w_sb[:, lo:hi],
                    in_=bass.AP(wmat, lo, [[NLINK * VX, 12], [1, hi - lo]]))

        loop_ctx = tc.For_i(0, repeat, 1) if repeat > 1 else None
        if loop_ctx is not None:
            load_w()  # constants stay resident across loop iterations
            ctx.enter_context(loop_ctx)

        hp_ctx = ExitStack()
        hp_ctx.enter_context(tc.high_priority(offset=HIPRI))

        # ---- band A: everything needed for the first output DMAs ----
        if loop_ctx is not None and KNOBS.get("warm_loop"):
            # re-warm the PE clock at the top of every loop iteration
            for _ in range(KNOBS["warm_loop"]):
                wps = psD.tile([P, P], F32, name="wps", space="PSUM")
                nc.tensor.transpose(wps, ident, ident)
        qp = qp_pool.tile([P, 4 * 26], F32, name="qp")
        nc.sync.dma_start(
            out=qp, in_=bass.AP(qpos, 0, [[26, P], [26 * P, 4], [1, 26]]))
        if loop_ctx is None:
            load_w()

        # joint sin/cos for all bt (feeds ACT early)
        xs = small.tile([P, 80], F32, name="xs")
        nc.vector.add_range_wrap(
            xs, _view(qp, 6, [(26, 4), (1, 20)]), 0.0, PI, TWO_PI)
        xc = small.tile([P, 80], F32, name="xc")
        nc.vector.add_range_wrap(
            xc, _view(qp, 6, [(26, 4), (1, 20)]), PI / 2, PI, TWO_PI)
        sinv = small.tile([P, 80], F32, name="sinv")
        nc.scalar.activation(sinv, xs, AF.Sin, bias=zero_c)
        cosv = small.tile([P, 80], F32, name="cosv")
        nc.scalar.activation(cosv, xc, AF.Sin, bias=zero_c)

        # root for bt0 only (critical path): s2 via one accumulating op,
        # a/b deg-4 Horner, R = b*(outer - s2*I) + I + a*K, all narrow ops
        COLS = ((1, 2, OP.subtract), (2, 1, OP.add),
                (4, 2, OP.add), (6, 0, OP.subtract),
                (8, 1, OP.subtract), (9, 0, OP.add))
        Tg = small.tile([P, 48], F32, name="Tg")

        s2_0 = small.tile([P, 1], F32, name="s2_0")
        sq3 = small.tile([P, 3], F32, name="sq3")
        nc.vector.scalar_tensor_tensor(
            sq3, _view(qp, 3, [(1, 3)]), 0.0, _view(qp, 3, [(1, 3)]),
            OP.add, OP.mult, accum_out=s2_0)
        y0 = small.tile([P, 2], F32, name="y0")
        nc.vector.tensor_copy(y0, _view(pt, 960, [(1, 2)]))
        for k in range(1, POLY_DEG + 1):
            nc.vector.tensor_mul(y0, y0, _view(s2_0, 0, [(0, 2)]))
            nc.vector.tensor_add(y0, y0, _view(pt, 960 + 2 * k, [(1, 2)]))
        outer0 = small.tile([P, 9], F32, name="outer0")
        nc.vector.tensor_mul(
            _view(outer0, 0, [(3, 3), (1, 3)]),
            _view(qp, 3, [(1, 3), (0, 3)]),
            _view(qp, 3, [(0, 3), (1, 3)]))
        nc.vector.tensor_tensor(
            _view(outer0, 0, [(4, 3)]), _view(outer0, 0, [(4, 3)]),
            _view(s2_0, 0, [(0, 3)]), OP.subtract)
        nc.vector.tensor_mul(
            _view(Tg, 0, [(4, 3), (1, 3)]),
            _view(outer0, 0, [(3, 3), (1, 3)]),
            _view(y0, 1, [(0, 3), (0, 3)]))
        nc.vector.tensor_scalar_add(
            _view(Tg, 0, [(5, 3)]), _view(Tg, 0, [(5, 3)]), 1.0)
        sa0 = small.tile([P, 3], F32, name="sa0")
        nc.vector.tensor_mul(sa0, _view(qp, 3, [(1, 3)]),
                             _view(y0, 0, [(0, 3)]))
        for (col, k, op) in COLS:
            v = _view(Tg, col, [(1, 1)])
            nc.vector.tensor_tensor(v, v, _view(sa0, k, [(1, 1)]), op)
        # Tg translation columns stay unwritten: j=0 composes read qp directly

        copy_i = 0

        def mk_copy(oslc, O_ps, pattern):
            nonlocal copy_i
            if pattern[copy_i % len(pattern)]:
                nc.vector.tensor_copy(oslc, O_ps)
            else:
                nc.scalar.copy(oslc, O_ps)
            copy_i += 1

        ident_b = const.tile([P, P], F32, name="ident_b") if KNOBS.get("gate") else None
        gate_state = {"done": False}

        def stage2_link(Tsrc, toff, bt, c, j, ot4, pattern, bank_dma):
            link = c * N_JOINTS + j
            idt = ident if (bt == 0 and c == 0) or ident_b is None else ident_b
            At_ps = psA.tile([12, P], F32, name="At_ps", space="PSUM")
            nc.tensor.transpose(At_ps, _view(Tsrc, toff, [(1, 12)]), idt)
            A12 = apool.tile([12, P], BF16, name="A12")
            if KNOBS["a12_eng"] == "scalar":
                nc.scalar.copy(A12, At_ps)
            else:
                nc.vector.tensor_copy(A12, At_ps)
            for i in range(3):
                O_ps = psO.tile([P, 512], F32, name="O_ps", space="PSUM")
                wv = _view(w_sb, link * VX + i * 512, [(1, 512)])
                nc.tensor.matmul(O_ps, A12[:, :], wv)
                if KNOBS.get("gate") and not gate_state["done"]:
                    # later transposes read ident_b, which depends on the
                    # first matmul: PE cannot run ahead hoarding transposes
                    nc.vector.scalar_tensor_tensor(
                        ident_b, _view(O_ps, 0, [(1, P)]), 0.0, ident,
                        OP.mult, OP.add)
                    gate_state["done"] = True
                oslc = ot4[:, j * VX + i * 512: j * VX + (i + 1) * 512]
                mk_copy(oslc, O_ps, pattern)
                if bank_dma:
                    dst = bass.AP(out, (bt * P) * ROW + link * VX + i * 512,
                                  [[ROW, P], [1, 512]])
                    nc.sync.dma_start(out=dst, in_=oslc)
            if not bank_dma and KNOBS["gran"] == "link":
                dst = bass.AP(out, (bt * P) * ROW + link * VX,
                              [[ROW, P], [1, VX]])
                nc.sync.dma_start(out=dst, in_=ot4[:, j * VX:(j + 1) * VX])

        def build_M(bt, j, sv_src, cv_src, eng):
            """M = P0 + s*P1 + c*P2 for all 5 chains of (bt, j), on `eng`."""
            M = mpool.tile([P, 80], F32, name="M")
            Mv = _view(M, 0, [(16, 5), (1, 16)])
            P0v = _view(pt, 16 * j, [(64, 5), (1, 16)])
            P1v = _view(pt, 320 + 16 * j, [(64, 5), (1, 16)])
            P2v = _view(pt, 640 + 16 * j, [(64, 5), (1, 16)])
            sv = _view(sv_src, 20 * bt + j, [(4, 5), (0, 16)])
            cv = _view(cv_src, 20 * bt + j, [(4, 5), (0, 16)])
            eng.tensor_mul(Mv, P1v, sv)
            eng.tensor_add(Mv, Mv, P0v)
            Mt = mpool.tile([P, 80], F32, name="Mt")
            Mtv = _view(Mt, 0, [(16, 5), (1, 16)])
            eng.tensor_mul(Mtv, P2v, cv)
            eng.tensor_add(Mv, Mv, Mtv)
            return M

        def compose_joint(bt, j, M, T_prev):
            T_new = tpool.tile([P, 60], F32, name="T_new")
            Tnv = _view(T_new, 0, [(12, 5), (4, 3), (1, 4)])
            Ttmp = tpool.tile([P, 60], F32, name="Ttmp", tag="Ttmp")
            Ttv = _view(Ttmp, 0, [(12, 5), (4, 3), (1, 4)])

            def prev_view(m):
                if T_prev is None:
                    return _view(Tg, bt * 12 + m, [(0, 5), (4, 3), (0, 4)])
                return _view(T_prev, m, [(12, 5), (4, 3), (0, 4)])

            def m_view(m):
                return _view(M, m * 4, [(16, 5), (0, 3), (1, 4)])

            nc.vector.tensor_mul(Tnv, prev_view(0), m_view(0))
            nc.vector.tensor_mul(Ttv, prev_view(1), m_view(1))
            nc.vector.tensor_add(Tnv, Tnv, Ttv)
            nc.vector.tensor_mul(Ttv, prev_view(2), m_view(2))
            nc.vector.tensor_add(Tnv, Tnv, Ttv)
            t3o = _view(T_new, 3, [(12, 5), (4, 3)])
            t3i = (_view(qp, bt * 26, [(0, 5), (1, 3)]) if T_prev is None
                   else _view(T_prev, 3, [(12, 5), (4, 3)]))
            nc.vector.tensor_tensor(t3o, t3o, t3i, OP.add)
            return T_new

        def stage2_bt(bt, T_list, first):
            for c in range(N_CHAINS):
                bank = first and c == 0 and KNOBS["c0gran"] == "bank"
                ot4 = ostage.tile([P, N_JOINTS * VX], F32, name="ot4")
                for j in range(N_JOINTS):
                    pat = KNOBS["pat_c0"] if (first and c == 0) else KNOBS["pat_rest"]
                    stage2_link(T_list[j], c * 12, bt, c, j, ot4, pat,
                                bank_dma=bank)
                if KNOBS["gran"] == "chain" and not bank:
                    dst = bass.AP(out, (bt * P) * ROW + c * N_JOINTS * VX,
                                  [[ROW, P], [1, N_JOINTS * VX]])
                    nc.sync.dma_start(out=dst, in_=ot4)

        # bt0: M on Pool (keeps DVE free for the root/compose chain)
        m_eng = nc.gpsimd if KNOBS["m_eng"] == "pool" else nc.vector
        T_prev = None
        T_list0 = []
        for j in range(N_JOINTS):
            M = build_M(0, j, sinv, cosv, m_eng)
            T_prev = compose_joint(0, j, M, T_prev)
            T_list0.append(T_prev)
        stage2_bt(0, T_list0, first=True)

        hp_ctx.close()  # leave high-priority band

        # ---- band B: bt1-3, gated on bt0's last compose so their DVE work
        # cannot interleave into the band-A critical chain ----
        z1 = small.tile([P, 1], F32, name="z1")
        t0v = _view(T_list0[3], 0, [(1, 1)])
        nc.gpsimd.tensor_tensor(z1, t0v, t0v, OP.subtract)  # exact 0, gated
        gs = small.tile([P, 80], F32, name="gs")
        nc.gpsimd.tensor_tensor(gs, sinv, _view(z1, 0, [(0, 80)]), OP.add)
        gc = small.tile([P, 80], F32, name="gc")
        nc.gpsimd.tensor_tensor(gc, cosv, _view(z1, 0, [(0, 80)]), OP.add)

        # bt1-3 root (3-wide), reading the gated qp copy
        qp_b = qp_pool.tile([P, 4 * 26], F32, name="qp_b")
        nc.gpsimd.tensor_tensor(qp_b, qp[:, :], _view(z1, 0, [(0, 104)]),
                                OP.add)
        sq9 = small.tile([P, 9], F32, name="sq9")
        nc.vector.tensor_mul(
            _view(sq9, 0, [(3, 3), (1, 3)]),
            _view(qp_b, 26 + 3, [(26, 3), (1, 3)]),
            _view(qp_b, 26 + 3, [(26, 3), (1, 3)]))
        s2_13 = small.tile([P, 3], F32, name="s2_13")
        nc.vector.tensor_reduce(
            s2_13, _view(sq9, 0, [(3, 3), (1, 3)]), AX.X, OP.add)
        y13 = small.tile([P, 6], F32, name="y13")
        nc.vector.tensor_copy(y13, _view(pt, 960, [(1, 2), (0, 3)]))
        for k in range(1, POLY_DEG + 1):
            nc.vector.tensor_mul(y13, y13, _view(s2_13, 0, [(0, 2), (1, 3)]))
            nc.vector.tensor_add(
                y13, y13, _view(pt, 960 + 2 * k, [(1, 2), (0, 3)]))
        outer13 = small.tile([P, 27], F32, name="outer13")
        nc.vector.tensor_mul(
            _view(outer13, 0, [(9, 3), (3, 3), (1, 3)]),
            _view(qp_b, 26 + 3, [(26, 3), (1, 3), (0, 3)]),
            _view(qp_b, 26 + 3, [(26, 3), (0, 3), (1, 3)]))
        nc.vector.tensor_tensor(
            _view(outer13, 0, [(9, 3), (4, 3)]),
            _view(outer13, 0, [(9, 3), (4, 3)]),
            _view(s2_13, 0, [(1, 3), (0, 3)]), OP.subtract)
        nc.vector.tensor_mul(
            _view(Tg, 12, [(12, 3), (4, 3), (1, 3)]),
            _view(outer13, 0, [(9, 3), (3, 3), (1, 3)]),
            _view(y13, 3, [(1, 3), (0, 3), (0, 3)]))
        nc.vector.tensor_scalar_add(
            _view(Tg, 12, [(12, 3), (5, 3)]),
            _view(Tg, 12, [(12, 3), (5, 3)]), 1.0)
        sa13 = small.tile([P, 9], F32, name="sa13")
        nc.vector.tensor_mul(
            _view(sa13, 0, [(3, 3), (1, 3)]),
            _view(qp_b, 26 + 3, [(26, 3), (1, 3)]),
            _view(y13, 0, [(1, 3), (0, 3)]))
        for (col, k, op) in COLS:
            v = _view(Tg, 12 + col, [(12, 3)])
            nc.vector.tensor_tensor(v, v, _view(sa13, k, [(3, 3)]), op)

        for bt in range(1, NB):
            T_prev = None
            T_list = []
            for j in range(N_JOINTS):
                M = build_M(bt, j, gs, gc, m_eng)
                T_prev = compose_joint(bt, j, M, T_prev)
                T_list.append(T_prev)
            stage2_bt(bt, T_list, first=False)

    nc.compile()
    return nc


_NC_CACHE = {}


def _get_nc(mm_mode=None, repeat=None):
    mm_mode = MM_MODE if mm_mode is None else mm_mode
    repeat = REPEAT if repeat is None else repeat
    key = (mm_mode, repeat, tuple(sorted((k, str(v)) for k, v in KNOBS.items())))
    if key not in _NC_CACHE:
        assert mm_mode == "bf16"
        _NC_CACHE[key] = _build_nc_bf16(repeat)
    return _NC_CACHE[key]


def _make_in_maps(qpos, offsets, axes, verts, mm_mode=None):
    import ml_dtypes
    qpos = np.ascontiguousarray(qpos, np.float32)
    pcon, W = _host_constants(np.asarray(offsets, np.float32),
                              np.asarray(axes, np.float32),
                              np.asarray(verts, np.float32))
    pcon_flat = np.ascontiguousarray(
        np.concatenate([pcon.reshape(-1), _sinc_coeffs()]))
    Wm = np.ascontiguousarray(
        W.reshape(12, NLINK * VX).astype(ml_dtypes.bfloat16))
    return [
        {"qpos": np.ascontiguousarray(qpos[i * B_CORE:(i + 1) * B_CORE]),
         "pcon": pcon_flat, "wmat": Wm}
        for i in range(N_CORES)
    ]


def kernel(qpos, offsets, axes, verts):
    nc = _get_nc()
    in_maps = _make_in_maps(qpos, offsets, axes, verts, MM_MODE)
    res = run_bass_kernel_spmd(nc, in_maps, core_ids=list(range(N_CORES)))
    outs = [res.results[i]["out"] for i in range(N_CORES)]
    full = np.concatenate(outs, axis=0)
    return full.reshape(B_FULL, N_CHAINS, N_JOINTS, N_VERTS, 3)
